# revision 1
# baseline (speedup 1.0000x reference)
"""BiDAF attention kernel for Trainium2, data-parallel over batch on 8 NeuronCores.

Math (per batch b, with w = [wc; wq; wm]):
    sim[i,j] = c_i@wc + q_j@wq + (c_i*wm)@q_j  =  cb_i + qb_j + s'[i,j]
    c2q      = softmax_j(sim) @ q
    q2c      = softmax_i(max_j sim) @ c            (broadcast over i)

Device strategy (softmax is shift-invariant per row, and |sim| <~ 12 so the
max-subtraction can be skipped entirely in fp32 range):
  - compute sim TRANSPOSED: simT[j,i] = sum_d kT[d,j] * cT[d,i]  (bf16 matmuls,
    fp32 PSUM accumulation) where kT = (q*wm)^T and cT = c^T come pre-transposed
    from the host.
  - ET[j,i] = exp(simT + qb[j])  via one ACT pass (qb is a per-partition bias).
    Then softmax_j(sim)[i,j] = ET[j,i] / S_i exactly (cb_i shift cancels).
  - c2q and the row-sums S_i in one matmul: [c2q_unnorm | S] = ET.T @ [q | 1],
    then scale by 1/S per partition.
  - row maxes: max_j ET[j,i] = exp(max_j(s'+qb)) by monotonicity. DVE max-tree
    over the 8 j-chunks, PE-transpose the [128,1024] result, reduce_max.
    e2_i = Emax_i * exp(cb_i) = exp(max_j sim[i,:]).
  - q2c_unnorm = c^T @ e2 on device; host divides by sum(e2) and broadcasts.
"""

import numpy as np

B, LC, LQ, D = 16, 1024, 1024, 256
N_CORES = 8
BPC = B // N_CORES  # batches per core

# number of c2q normalize copies routed to ACT (rest go to DVE) per batch
_NORM_ON_ACT = 8
# bf16 on the averaging paths (ET, qa, cn, e2): halves their DMA + DVE cost;
# sim stays fp32r and c2q output stays fp32. Adds ~2e-3 absmax error.
_AVG_BF16 = True
# bf16 for the similarity matmul inputs too (cT, kT): halves their DMA.
# Total error ~4e-3 absmax / ~7e-6 resid_var.
_SIM_BF16 = True
# bf16 c2q output (host upcasts): saves 1MB/core of store traffic.
_OUT_BF16 = True

_CACHE = {}


def build_program(repeat_inner=1, n_cores=N_CORES, ablate=()):
    """Build + compile the SPMD bass program (one core's view, BPC batches).

    repeat_inner > 1 repeats the whole body (for timing amplification)."""
    import concourse.bacc as bacc
    import concourse.tile as tile
    from concourse import mybir

    f32 = mybir.dt.float32
    f32r = mybir.dt.float32r
    bf16 = mybir.dt.bfloat16
    avg_dt = bf16 if _AVG_BF16 else f32r
    sim_dt = bf16 if _SIM_BF16 else f32r

    nc = bacc.Bacc(
        "TRN2",
        target_bir_lowering=False,
        debug=False,
        enable_asserts=False,
        num_devices=n_cores,
    )

    # DRAM I/O (per-core shapes)
    cT_d = nc.dram_tensor("ct", [BPC, D, LC], sim_dt, kind="ExternalInput").ap()
    kT_d = nc.dram_tensor("kt", [BPC, D, LQ], sim_dt, kind="ExternalInput").ap()
    qa_d = nc.dram_tensor("qa", [BPC, LQ, D + 2], avg_dt, kind="ExternalInput").ap()
    cn_d = nc.dram_tensor("cn", [BPC, LC, D], avg_dt, kind="ExternalInput").ap()
    qb_d = nc.dram_tensor("qb", [BPC, 128, LQ // 128], f32, kind="ExternalInput").ap()
    gcb_d = nc.dram_tensor("gcb", [BPC, 128, LC // 128], avg_dt, kind="ExternalInput").ap()
    id_d = nc.dram_tensor("ident", [128, 128], avg_dt, kind="ExternalInput").ap()

    out_dt = bf16 if _OUT_BF16 else f32
    c2q_d = nc.dram_tensor("c2q", [BPC, LC, D], out_dt, kind="ExternalOutput").ap()
    q2cu_d = nc.dram_tensor("q2cu", [BPC, 128, D // 128], f32, kind="ExternalOutput").ap()
    e2_d = nc.dram_tensor("e2", [BPC, 128, LC // 128], avg_dt, kind="ExternalOutput").ap()

    NJ = LQ // 128  # 8 j-chunks
    NI = LC // 128  # 8 i-chunks
    ND = D // 128   # 2 d-chunks

    with tile.TileContext(nc) as tc:
        with (
            tc.tile_pool(name="const", bufs=1) as const_pool,
            tc.tile_pool(name="io", bufs=2) as io_pool,
            tc.tile_pool(name="et", bufs=2) as et_pool,
            tc.tile_pool(name="tree", bufs=4) as tree_pool,
            tc.tile_pool(name="small", bufs=4) as small_pool,
            tc.tile_pool(name="outs", bufs=4) as out_pool,
            tc.tile_pool(name="psum_sim", bufs=2, space="PSUM") as sim_pool,
            tc.tile_pool(name="psum_c2q", bufs=2, space="PSUM") as c2q_pool,
            tc.tile_pool(name="psum_tr", bufs=2, space="PSUM") as tr_pool,
        ):
            ident = const_pool.tile([128, 128], avg_dt)
            nc.sync.dma_start(ident[:], id_d[:])

            for _rep in range(repeat_inner):
                for b in range(BPC):
                    # ---- load inputs in first-consumer order: the first sim
                    # matmul needs only kT[d0, j0-tile] and cT[d0] ----
                    kT_s = io_pool.tile([128, ND, LQ], sim_dt, tag="kt")
                    cT_s = io_pool.tile([128, ND, LC], sim_dt, tag="ct")
                    kT_r = kT_d[b].rearrange("(c p) n -> p c n", p=128)
                    cT_r = cT_d[b].rearrange("(c p) n -> p c n", p=128)
                    nc.sync.dma_start(kT_s[:, 0:1, 0:128], kT_r[:, 0:1, 0:128])
                    nc.sync.dma_start(cT_s[:, 0:1, :], cT_r[:, 0:1, :])
                    qb_s = io_pool.tile([128, NJ], f32, tag="qb")
                    nc.sync.dma_start(qb_s[:], qb_d[b])
                    gcb_s = io_pool.tile([128, NI], avg_dt, tag="gcb")
                    nc.sync.dma_start(gcb_s[:], gcb_d[b])
                    nc.sync.dma_start(kT_s[:, 1:2, 0:128], kT_r[:, 1:2, 0:128])
                    nc.sync.dma_start(cT_s[:, 1:2, :], cT_r[:, 1:2, :])
                    nc.sync.dma_start(kT_s[:, 0:1, 128:LQ], kT_r[:, 0:1, 128:LQ])
                    nc.sync.dma_start(kT_s[:, 1:2, 128:LQ], kT_r[:, 1:2, 128:LQ])

                    qa_s = io_pool.tile([128, NJ, D + 2], avg_dt, tag="qa")
                    qa_r = qa_d[b].rearrange("(c p) n -> p c n", p=128)
                    nc.sync.dma_start(qa_s[:, 0:2, :], qa_r[:, 0:2, :])
                    nc.sync.dma_start(qa_s[:, 2:NJ, :], qa_r[:, 2:NJ, :])
                    cn_s = io_pool.tile([128, NI, D], avg_dt, tag="cn")
                    nc.sync.dma_start(
                        cn_s[:], cn_d[b].rearrange("(c p) n -> p c n", p=128))

                    # ---- simT + exp + running max over j-chunks ----
                    # ET[j, jc, i] = exp(sim[i, jc*128+j] - cb_i)
                    ET = et_pool.tile([128, NJ, LQ], avg_dt, tag="et")
                    mx = tree_pool.tile([128, LC], avg_dt, tag="mx")
                    for jt in range(NJ):
                        ps = sim_pool.tile([128, LC], f32, tag="sim")
                        if "sim" not in ablate:
                            for nh in range(2):
                                cols = slice(nh * 512, (nh + 1) * 512)
                                for dc in range(ND):
                                    nc.tensor.matmul(
                                        ps[:, cols],
                                        lhsT=kT_s[:, dc, jt * 128:(jt + 1) * 128],
                                        rhs=cT_s[:, dc, cols],
                                        start=(dc == 0),
                                        stop=(dc == ND - 1),
                                    )
                        if "exp" not in ablate:
                            nc.scalar.activation(
                                ET[:, jt, :], ps[:],
                                mybir.ActivationFunctionType.Exp,
                                bias=qb_s[:, jt:jt + 1], scale=1.0,
                            )
                        if "max" not in ablate:
                            if jt == 1:
                                nc.vector.tensor_max(mx[:], ET[:, 0, :], ET[:, 1, :])
                            elif jt > 1:
                                nc.vector.tensor_max(mx[:], mx[:], ET[:, jt, :])

                    # ---- c2q (+ row sums via the ones column of qa) ----
                    for ic in range(NI if "c2q" not in ablate else 0):
                        pc = c2q_pool.tile([128, D + 2], f32, tag="c2q")
                        for jc in range(NJ):
                            nc.tensor.matmul(
                                pc[:],
                                lhsT=ET[:, jc, ic * 128:(ic + 1) * 128],
                                rhs=qa_s[:, jc, :],
                                start=(jc == 0),
                                stop=(jc == NJ - 1),
                            )
                        rs = small_pool.tile([128, 1], f32, tag="recip")
                        nc.vector.reciprocal(rs[:], pc[:, D:D + 1])
                        ot = out_pool.tile([128, D], out_dt, tag="c2qo")
                        if ic < _NORM_ON_ACT:
                            nc.scalar.mul(ot[:], pc[:, 0:D], rs[:])
                        else:
                            nc.vector.tensor_scalar_mul(ot[:], pc[:, 0:D], rs[:])
                        nc.sync.dma_start(c2q_d[b, ic * 128:(ic + 1) * 128, :], ot[:])

                    # ---- finish row maxes: PE transpose + free-dim reduce ----
                    e2_s = out_pool.tile([128, NI + 1], avg_dt, tag="e2")
                    nc.vector.memset(e2_s[:, NI:NI + 1], 0.0)
                    for ic in range(NI if "max" not in ablate else 0):
                        tr = tr_pool.tile([128, 128], avg_dt, tag="tr")
                        nc.tensor.transpose(
                            tr[:], mx[:, ic * 128:(ic + 1) * 128], ident[:])
                        em = small_pool.tile([128, 1], avg_dt, tag="emax")
                        nc.vector.reduce_max(
                            out=em[:], in_=tr[:], axis=mybir.AxisListType.X)
                        nc.vector.tensor_mul(
                            e2_s[:, ic:ic + 1], em[:], gcb_s[:, ic:ic + 1])
                    nc.sync.dma_start(e2_d[b], e2_s[:, 0:NI])

                    # ---- q2c numerator: sum_i e2_i * c[i, :] ----
                    pq = c2q_pool.tile([128, D + 2], f32, tag="c2q")
                    for dc in range(ND if "q2c" not in ablate else 0):
                        for ic in range(NI):
                            nc.tensor.matmul(
                                pq[:, dc:dc + 1],
                                lhsT=cn_s[:, ic, dc * 128:(dc + 1) * 128],
                                rhs=e2_s[:, ic:ic + 1],
                                start=(ic == 0),
                                stop=(ic == NI - 1),
                            )
                    qo = out_pool.tile([128, ND], f32, tag="q2co")
                    nc.vector.tensor_copy(qo[:], pq[:, 0:ND])
                    nc.sync.dma_start(q2cu_d[b], qo[:])

    nc.compile()
    return nc


def _host_prep(context_features, question_features, weight):
    c = np.ascontiguousarray(context_features, dtype=np.float32)
    q = np.ascontiguousarray(question_features, dtype=np.float32)
    w = np.asarray(weight, dtype=np.float32)[:, 0]
    wc, wq, wm = w[:D], w[D:2 * D], w[2 * D:]

    qb = q @ wq                       # [B, LQ]
    cb = c @ wc                       # [B, LC]
    gcb = np.exp(cb)                  # [B, LC]

    cT = np.ascontiguousarray(c.transpose(0, 2, 1))            # [B, D, LC]
    kT = np.ascontiguousarray((q * wm).transpose(0, 2, 1))     # [B, D, LQ]
    if _SIM_BF16:
        import ml_dtypes
        cT = cT.astype(ml_dtypes.bfloat16)
        kT = kT.astype(ml_dtypes.bfloat16)
    qa = np.concatenate(
        [q, np.ones((B, LQ, 1), np.float32),
         np.zeros((B, LQ, 1), np.float32)], axis=2)            # [B, LQ, D+2]
    if _AVG_BF16:
        import ml_dtypes
        bf = ml_dtypes.bfloat16
        qa = qa.astype(bf)
        cn_h = c.astype(bf)
        gcb_cast = gcb.astype(bf)
    else:
        cn_h = c
        gcb_cast = gcb

    qb_t = np.ascontiguousarray(
        qb.reshape(B, LQ // 128, 128).transpose(0, 2, 1))      # [B, 128, 8]
    gcb_t = np.ascontiguousarray(
        gcb_cast.reshape(B, LC // 128, 128).transpose(0, 2, 1))  # [B, 128, 8]

    if _AVG_BF16:
        import ml_dtypes
        ident = np.eye(128, dtype=ml_dtypes.bfloat16)
    else:
        ident = np.eye(128, dtype=np.float32)

    in_maps = []
    for core in range(N_CORES):
        s = slice(core * BPC, (core + 1) * BPC)
        in_maps.append({
            "ct": cT[s], "kt": kT[s], "qa": qa[s], "cn": cn_h[s],
            "qb": qb_t[s], "gcb": gcb_t[s], "ident": ident,
        })
    return in_maps


def _assemble(results):
    c2q = np.concatenate(
        [np.asarray(r["c2q"], dtype=np.float32) for r in results], axis=0)
    q2cu = np.concatenate([r["q2cu"] for r in results], axis=0)    # [B, 128, 2]
    e2 = np.concatenate(
        [np.asarray(r["e2"], dtype=np.float32) for r in results], axis=0)

    s2 = e2.sum(axis=(1, 2))                                       # [B]
    q2c_vec = q2cu.transpose(0, 2, 1).reshape(B, D) / s2[:, None]  # [B, D]
    q2c = np.broadcast_to(q2c_vec[:, None, :], (B, LC, D)).copy()
    return c2q, q2c


def _make_runner(nc, n_cores):
    """Compile the bass program once into a reusable sharded jax callable.

    Mirrors concourse.bass2jax.run_bass_via_pjrt, but returns a cached
    executable so repeated kernel() calls don't re-trace/re-compile."""
    import jax
    import numpy as np
    from jax.sharding import Mesh, PartitionSpec
    from jax.experimental.shard_map import shard_map
    from concourse import mybir
    from concourse.bass2jax import (
        _bass_exec_p, install_neuronx_cc_hook, partition_id_tensor)

    install_neuronx_cc_hook()

    partition_name = nc.partition_id_tensor.name if nc.partition_id_tensor else None
    in_names, out_names, out_avals, zero_shapes = [], [], [], []
    for alloc in nc.m.functions[0].allocations:
        if not isinstance(alloc, mybir.MemoryLocationSet):
            continue
        name = alloc.memorylocations[0].name
        if alloc.kind == "ExternalInput":
            if name != partition_name:
                in_names.append(name)
        elif alloc.kind == "ExternalOutput":
            out_names.append(name)
            shape = tuple(alloc.tensor_shape)
            dtype = mybir.dt.np(alloc.dtype)
            out_avals.append(jax.core.ShapedArray(shape, dtype))
            zero_shapes.append((shape, dtype))
    n_params = len(in_names)
    all_names = list(in_names) + list(out_names)
    if partition_name is not None:
        all_names.append(partition_name)

    def _body(*args):
        operands = list(args)
        if partition_name is not None:
            operands.append(partition_id_tensor())
        outs = _bass_exec_p.bind(
            *operands,
            out_avals=tuple(out_avals),
            in_names=tuple(all_names),
            out_names=tuple(out_names),
            lowering_input_output_aliases=(),
            sim_require_finite=True,
            sim_require_nnan=True,
            nc=nc,
        )
        return tuple(outs)

    devices = jax.devices()[:n_cores]
    assert len(devices) == n_cores, f"need {n_cores} cores"
    mesh = Mesh(np.asarray(devices), ("core",))
    n_outs = len(out_names)
    fn = jax.jit(
        shard_map(
            _body, mesh=mesh,
            in_specs=(PartitionSpec("core"),) * (n_params + n_outs),
            out_specs=(PartitionSpec("core"),) * n_outs,
            check_rep=False),
        keep_unused=True,
    )
    sharding = jax.sharding.NamedSharding(mesh, PartitionSpec("core"))
    zeros = [
        jax.device_put(
            np.zeros((shape[0] * n_cores,) + tuple(shape[1:]), dtype), sharding)
        for shape, dtype in zero_shapes
    ]

    def run(in_maps):
        concat_in = [
            np.concatenate([np.asarray(m[name]) for m in in_maps], axis=0)
            for name in in_names
        ]
        dev_in = [jax.device_put(a, sharding) for a in concat_in]
        outs = fn(*dev_in, *zeros)
        results = []
        for c in range(n_cores):
            d = {}
            for name, arr in zip(out_names, outs):
                arr = np.asarray(arr)
                per = arr.shape[0] // n_cores
                d[name] = arr[c * per:(c + 1) * per]
            results.append(d)
        return results

    return run


def kernel(context_features, question_features, weight):
    if "run" not in _CACHE:
        nc = build_program()
        _CACHE["nc"] = nc
        _CACHE["run"] = _make_runner(nc, N_CORES)

    in_maps = _host_prep(context_features, question_features, weight)
    results = _CACHE["run"](in_maps)
    c2q, q2c = _assemble(results)
    return c2q, q2c



# revision 2
# speedup vs baseline: 1.2026x; 1.2026x over previous
"""BiDAF attention kernel v3 for Trainium2 (bf16 core, lean engines).

Device work per batch (PE does ONLY the two essential matmuls):
  - simT psum[j,i] = kT.T @ cT  (bf16, K=256 via 2-matmul groups)
  - ET[j, jt, i] = exp(psum + qb_j) via ACT (bf16 out), qb bias per partition
  - c2q numerator + row sums S: [num | S] = ET.T @ [q | 1 | 0]  (bf16)
    PSUM -> SBUF f32 copies on DVE, one DMA out; HOST divides num/S.
  - row maxes: max tree over jt chunks (l1 x4 on Pool, l2/l3 on DVE),
    SBUF->SBUF DMA-transpose of mx, one DVE reduce_max -> Emax bf16 out.
    HOST: e2 = Emax * exp(cb); q2c = (e2/sum) @ c; broadcast over Lc.

Gone vs the old kernel: PE transposes, on-device q2c matmuls, ACT normalize
muls, DVE reciprocals, gcb/cn/ident inputs.
"""

import numpy as np

B, LC, LQ, D = 16, 1024, 1024, 256
N_CORES = 8
BPC = B // N_CORES
NJ = LQ // 128
NI = LC // 128

_CACHE = {}


def build_program(repeat_inner=1, n_cores=N_CORES, ablate=(), loop_n=None):
    """One core's program (BPC batches per repeat). If loop_n is given, wrap
    the whole repeated body in a hardware For_i loop (timing amplification)."""
    import concourse.bacc as bacc
    import concourse.tile as tile
    from concourse import mybir
    from contextlib import nullcontext

    f32 = mybir.dt.float32
    bf16 = mybir.dt.bfloat16

    nc = bacc.Bacc(
        "TRN2",
        target_bir_lowering=False,
        debug=False,
        enable_asserts=False,
        num_devices=n_cores,
    )

    kT_d = nc.dram_tensor("kt", [BPC, D, LQ], bf16, kind="ExternalInput").ap()
    cT_d = nc.dram_tensor("ct", [BPC, D, LC], bf16, kind="ExternalInput").ap()
    qa_d = nc.dram_tensor("qa", [BPC, LQ, D + 2], bf16, kind="ExternalInput").ap()
    qb_d = nc.dram_tensor("qb", [BPC, 128, NJ], f32, kind="ExternalInput").ap()

    c2qu_d = nc.dram_tensor(
        "c2qu", [BPC, NI, 128, D + 2], bf16, kind="ExternalOutput").ap()
    emax_d = nc.dram_tensor(
        "emax", [BPC, 128, NI], bf16, kind="ExternalOutput").ap()

    with tile.TileContext(nc) as tc:
        with (
            tc.tile_pool(name="io", bufs=3) as io_pool,
            tc.tile_pool(name="et", bufs=2) as et_pool,
            tc.tile_pool(name="tree", bufs=2) as tree_pool,
            tc.tile_pool(name="outs", bufs=2) as out_pool,
            tc.tile_pool(name="psum_sim", bufs=3, space="PSUM") as sim_pool,
            tc.tile_pool(name="psum_c2q", bufs=2, space="PSUM") as c2q_pool,
        ):
            loop_cm = tc.For_i(0, loop_n, 1) if loop_n is not None else nullcontext()
            with loop_cm:
                for _rep in range(repeat_inner):
                    for b in range(BPC):
                        kT_s = io_pool.tile([128, 2, LQ], bf16, tag="kt")
                        cT_s = io_pool.tile([128, 2, LC], bf16, tag="ct")
                        kT_r = kT_d[b].rearrange("(c p) n -> p c n", p=128)
                        cT_r = cT_d[b].rearrange("(c p) n -> p c n", p=128)
                        nc.sync.dma_start(kT_s[:, :, 0:128], kT_r[:, :, 0:128])
                        nc.sync.dma_start(cT_s[:, :, 0:512], cT_r[:, :, 0:512])
                        qb_s = io_pool.tile([128, NJ], f32, tag="qb")
                        nc.sync.dma_start(qb_s[:], qb_d[b])
                        nc.sync.dma_start(cT_s[:, :, 512:LC], cT_r[:, :, 512:LC])
                        nc.sync.dma_start(kT_s[:, :, 128:LQ], kT_r[:, :, 128:LQ])
                        qa_s = io_pool.tile([128, NJ, D + 2], bf16, tag="qa")
                        nc.sync.dma_start(
                            qa_s[:], qa_d[b].rearrange("(c p) n -> p c n", p=128))

                        # ---- simT + exp (full-jt, 3-deep psum) ----
                        ET = et_pool.tile([128, NJ, LC], bf16, tag="et")
                        for jt in range(NJ):
                            ps = sim_pool.tile([128, LC], f32, tag="sim")
                            if "sim" not in ablate:
                                for nh in range(2):
                                    cols = slice(nh * 512, (nh + 1) * 512)
                                    for dc in range(2):
                                        nc.tensor.matmul(
                                            ps[:, cols],
                                            lhsT=kT_s[:, dc,
                                                      jt * 128:(jt + 1) * 128],
                                            rhs=cT_s[:, dc, cols],
                                            start=(dc == 0), stop=(dc == 1),
                                        )
                            if "exp" not in ablate:
                                nc.scalar.activation(
                                    ET[:, jt, :], ps[:],
                                    mybir.ActivationFunctionType.Exp,
                                    bias=qb_s[:, jt:jt + 1], scale=1.0,
                                )

                        # ---- row maxes (DVE tree; aux DMAs on ACT) ----
                        if "max" not in ablate:
                            t1 = tree_pool.tile([128, 4, LC], bf16, tag="t1")
                            for u in range(4):
                                nc.vector.tensor_max(
                                    t1[:, u, :], ET[:, 2 * u, :],
                                    ET[:, 2 * u + 1, :])
                            t2 = tree_pool.tile([128, 2, LC], bf16, tag="t2")
                            nc.vector.tensor_max(
                                t2[:, 0, :], t1[:, 0, :], t1[:, 1, :])
                            nc.vector.tensor_max(
                                t2[:, 1, :], t1[:, 2, :], t1[:, 3, :])
                            mx = tree_pool.tile([128, LC], bf16, tag="mx")
                            nc.vector.tensor_max(mx[:], t2[:, 0, :], t2[:, 1, :])
                            mxT = tree_pool.tile([128, NI, 128], bf16, tag="mxT")
                            nc.scalar.dma_start_transpose(mxT[:], mx[:])
                            emax_s = out_pool.tile([128, NI, 1], bf16, tag="emax")
                            nc.vector.reduce_max(
                                out=emax_s[:], in_=mxT[:],
                                axis=mybir.AxisListType.X)
                            nc.scalar.dma_start(emax_d[b], emax_s[:, :, 0])
                        # ---- c2q numerator + S (psum copies on ACT) ----
                        c2qo = out_pool.tile([128, NI, D + 2], bf16, tag="c2qo")
                        for ic in range(NI if "c2q" not in ablate else 0):
                            pc = c2q_pool.tile([128, D + 2], f32, tag="c2q")
                            for jc in range(NJ):
                                nc.tensor.matmul(
                                    pc[:],
                                    lhsT=ET[:, jc, ic * 128:(ic + 1) * 128],
                                    rhs=qa_s[:, jc, :],
                                    start=(jc == 0), stop=(jc == NJ - 1),
                                )
                            nc.vector.tensor_copy(c2qo[:, ic, :], pc[:])
                        if "c2q" not in ablate:
                            nc.scalar.dma_start(
                                c2qu_d[b].rearrange("c p n -> p c n", p=128),
                                c2qo[:])

    nc.compile()
    return nc


def _host_prep(context_features, question_features, weight):
    import ml_dtypes
    BF = ml_dtypes.bfloat16

    c = np.ascontiguousarray(context_features, dtype=np.float32)
    q = np.ascontiguousarray(question_features, dtype=np.float32)
    w = np.asarray(weight, dtype=np.float32)[:, 0]
    wc, wq, wm = w[:D], w[D:2 * D], w[2 * D:]

    qb = (q @ wq).astype(np.float32)                   # [B, LQ]
    cb = c @ wc                                        # [B, LC] (host only)

    kT = np.ascontiguousarray(
        (q * wm).transpose(0, 2, 1)).astype(BF)        # [B, D, LQ]
    cT = np.ascontiguousarray(c.transpose(0, 2, 1)).astype(BF)
    qa = np.concatenate(
        [q, np.ones((B, LQ, 1), np.float32),
         np.zeros((B, LQ, 1), np.float32)], axis=2).astype(BF)

    qb_t = np.ascontiguousarray(
        qb.reshape(B, NJ, 128).transpose(0, 2, 1))     # [B, 128, NJ]

    in_maps = []
    for core in range(N_CORES):
        s = slice(core * BPC, (core + 1) * BPC)
        in_maps.append({
            "kt": kT[s], "ct": cT[s], "qa": qa[s], "qb": qb_t[s],
        })
    _CACHE["cb"] = cb
    _CACHE["c"] = c
    return in_maps


def _assemble(results):
    c, cb = _CACHE["c"], _CACHE["cb"]
    c2qu = np.concatenate(
        [np.asarray(r["c2qu"], dtype=np.float32) for r in results], axis=0)
    emax = np.concatenate(
        [np.asarray(r["emax"], dtype=np.float32) for r in results], axis=0)

    num = c2qu[..., 0:D]
    S = c2qu[..., D:D + 1]
    c2q = (num / S).reshape(B, LC, D)

    em = emax.transpose(0, 2, 1).reshape(B, LC)          # [B, LC]
    e2 = em * np.exp(cb)
    wgt = e2 / e2.sum(axis=1, keepdims=True)
    q2c_vec = np.einsum('bc,bcd->bd', wgt, c)
    q2c = np.broadcast_to(q2c_vec[:, None, :], (B, LC, D)).copy()
    return c2q.astype(np.float32), q2c.astype(np.float32)


def _make_runner(nc, n_cores):
    import jax
    from jax.sharding import Mesh, PartitionSpec
    from jax.experimental.shard_map import shard_map
    from concourse import mybir
    from concourse.bass2jax import (
        _bass_exec_p, install_neuronx_cc_hook, partition_id_tensor)

    install_neuronx_cc_hook()

    partition_name = nc.partition_id_tensor.name if nc.partition_id_tensor else None
    in_names, out_names, out_avals, zero_shapes = [], [], [], []
    for alloc in nc.m.functions[0].allocations:
        if not isinstance(alloc, mybir.MemoryLocationSet):
            continue
        name = alloc.memorylocations[0].name
        if alloc.kind == "ExternalInput":
            if name != partition_name:
                in_names.append(name)
        elif alloc.kind == "ExternalOutput":
            out_names.append(name)
            shape = tuple(alloc.tensor_shape)
            dtype = mybir.dt.np(alloc.dtype)
            out_avals.append(jax.core.ShapedArray(shape, dtype))
            zero_shapes.append((shape, dtype))
    n_params = len(in_names)
    all_names = list(in_names) + list(out_names)
    if partition_name is not None:
        all_names.append(partition_name)

    def _body(*args):
        operands = list(args)
        if partition_name is not None:
            operands.append(partition_id_tensor())
        outs = _bass_exec_p.bind(
            *operands,
            out_avals=tuple(out_avals),
            in_names=tuple(all_names),
            out_names=tuple(out_names),
            lowering_input_output_aliases=(),
            sim_require_finite=True,
            sim_require_nnan=True,
            nc=nc,
        )
        return tuple(outs)

    devices = jax.devices()[:n_cores]
    assert len(devices) == n_cores, f"need {n_cores} cores"
    mesh = Mesh(np.asarray(devices), ("core",))
    n_outs = len(out_names)
    fn = jax.jit(
        shard_map(
            _body, mesh=mesh,
            in_specs=(PartitionSpec("core"),) * (n_params + n_outs),
            out_specs=(PartitionSpec("core"),) * n_outs,
            check_rep=False),
        keep_unused=True,
    )
    sharding = jax.sharding.NamedSharding(mesh, PartitionSpec("core"))
    zeros = [
        jax.device_put(
            np.zeros((shape[0] * n_cores,) + tuple(shape[1:]), dtype), sharding)
        for shape, dtype in zero_shapes
    ]

    def run(in_maps):
        concat_in = [
            np.concatenate([np.asarray(m[name]) for m in in_maps], axis=0)
            for name in in_names
        ]
        dev_in = [jax.device_put(a, sharding) for a in concat_in]
        outs = fn(*dev_in, *zeros)
        results = []
        for cix in range(n_cores):
            d = {}
            for name, arr in zip(out_names, outs):
                arr = np.asarray(arr)
                per = arr.shape[0] // n_cores
                d[name] = arr[cix * per:(cix + 1) * per]
            results.append(d)
        return results

    return run


def kernel(context_features, question_features, weight):
    if "run" not in _CACHE:
        nc = build_program()
        _CACHE["nc"] = nc
        _CACHE["run"] = _make_runner(nc, N_CORES)

    in_maps = _host_prep(context_features, question_features, weight)
    results = _CACHE["run"](in_maps)
    return _assemble(results)
